# revision 2
# baseline (speedup 1.0000x reference)
# Contrastive-loss kernel for Trainium2 (Bass/Tile), 8-core data-parallel.
#
# Math (see reference):
#   S[i,j]     = (x_i . y_j) / T
#   denom[i,k] = sum_{j<=k} exp(S[i,j]) + (B-1-k)
#   loss       = sum_{i,k} log(denom[i,k]) - sum_i (B-i) * S[i,i]
#
# Device formulation per core (512 rows of x, full y):
#   - matmul (bf16) -> PSUM S_raw tiles [128, 512]
#   - ACT exp with scale=1/T : expS = exp(S_raw/T)            (PSUM -> SBUF)
#   - DVE tensor_tensor_scan: denom[k] = B + cumsum(expS - 1)
#         state = (expS[k] + state) + (-1), initial = B
#     (identical to cumE[k] + (B-1-k))
#   - ACT ln with accum_out: per-partition sum_k log(denom)
#   - diag: partial[p] = lnsum[p] + sum_d(xpre ⊙ y_row)   with
#         xpre = -(B-i)/T * x  (host-precomputed)  == lnsum - (B-i)*S_ii
#   - host sums the 8 x [128, 4] partials -> scalar loss.

import numpy as np
import ml_dtypes

B = 4096
D = 256
NCORES = 8
ROWS = B // NCORES      # 512 rows per core
P = 128                 # SBUF partitions
RT = ROWS // P          # 4 row-tiles per core
JT = 512                # matmul moving free-dim tile
HALF = 2048             # psum/exp chunk (4 banks)
TEMP = 0.07

_CACHE = {}
LAST_RESULTS = None     # BassKernelResults of the most recent run (for test.py)


def _build():
    from contextlib import ExitStack

    import concourse.bacc as bacc
    import concourse.mybir as mybir
    import concourse.tile as tile

    dt = mybir.dt
    Act = mybir.ActivationFunctionType
    Alu = mybir.AluOpType

    nc = bacc.Bacc(
        "TRN2", target_bir_lowering=False, debug=False, num_devices=NCORES
    )

    xT = nc.dram_tensor("xT", (D, ROWS), dt.bfloat16, kind="ExternalInput").ap()
    yT = nc.dram_tensor("yT", (D, B), dt.bfloat16, kind="ExternalInput").ap()
    xpre = nc.dram_tensor("xpre", (ROWS, D), dt.float32, kind="ExternalInput").ap()
    ysh = nc.dram_tensor("ysh", (ROWS, D), dt.float32, kind="ExternalInput").ap()
    out = nc.dram_tensor("partial", (P, RT), dt.float32, kind="ExternalOutput").ap()

    with tile.TileContext(nc) as tc, ExitStack() as ctx:
        wpool = ctx.enter_context(tc.tile_pool(name="weights", bufs=1))
        psum = ctx.enter_context(tc.tile_pool(name="psum", bufs=2, space="PSUM"))
        big = ctx.enter_context(tc.tile_pool(name="big", bufs=2))
        small = ctx.enter_context(tc.tile_pool(name="small", bufs=4))

        # x^T shard: two K-chunks of [128, 512] bf16
        xT_t = []
        for kc in range(2):
            xt = wpool.tile([P, ROWS], dt.bfloat16, name=f"xTs{kc}")
            nc.sync.dma_start(out=xt, in_=xT[kc * P:(kc + 1) * P, :])
            xT_t.append(xt)
        # y^T: two K-chunks of [128, 4096] bf16; split loads so DMA queues
        # parallelize and matmuls can start on the first column chunk.
        yT_t = []
        for kc in range(2):
            yt = wpool.tile([P, B], dt.bfloat16, name=f"yTs{kc}")
            yT_t.append(yt)
        for j in range(4):
            for kc in range(2):
                nc.sync.dma_start(
                    out=yT_t[kc][:, j * 1024:(j + 1) * 1024],
                    in_=yT[kc * P:(kc + 1) * P, j * 1024:(j + 1) * 1024],
                )

        negones = wpool.tile([P, B], dt.float32)
        nc.vector.memset(negones, -1.0)

        resall = wpool.tile([P, RT], dt.float32)

        for m in range(RT):
            expS = big.tile([P, B], dt.float32, tag="expS")
            for h in range(2):
                ps = psum.tile([P, HALF], dt.float32, tag="ps")
                for jb in range(HALF // JT):
                    j0 = h * HALF + jb * JT
                    for kc in range(2):
                        nc.tensor.matmul(
                            ps[:, jb * JT:(jb + 1) * JT],
                            xT_t[kc][:, m * P:(m + 1) * P],
                            yT_t[kc][:, j0:j0 + JT],
                            start=(kc == 0),
                            stop=(kc == 1),
                        )
                nc.scalar.activation(
                    out=expS[:, h * HALF:(h + 1) * HALF],
                    in_=ps,
                    func=Act.Exp,
                    scale=1.0 / TEMP,
                )

            denom = big.tile([P, B], dt.float32, tag="denom")
            nc.vector.tensor_tensor_scan(
                out=denom,
                data0=expS,
                data1=negones,
                initial=float(B),
                op0=Alu.add,
                op1=Alu.add,
            )

            lsum = small.tile([P, 1], dt.float32, tag="lsum")
            # ln output overwrites expS (dead after the scan); accum_out is
            # the per-partition sum of log(denom).
            nc.scalar.activation(
                out=expS, in_=denom, func=Act.Ln, accum_out=lsum
            )

            #

            xp = small.tile([P, D], dt.float32, tag="xp")
            nc.sync.dma_start(out=xp, in_=xpre[m * P:(m + 1) * P, :])
            yp = small.tile([P, D], dt.float32, tag="yp")
            nc.sync.dma_start(out=yp, in_=ysh[m * P:(m + 1) * P, :])
            prod = small.tile([P, D], dt.float32, tag="prod")
            dterm = small.tile([P, 1], dt.float32, tag="dterm")
            # dterm = sum_d(xpre * y) = -(B-i)*S_ii  (xpre negated on host)
            nc.vector.scalar_tensor_tensor(
                out=prod,
                in0=xp,
                scalar=1.0,
                in1=yp,
                op0=Alu.mult,
                op1=Alu.mult,
                accum_out=dterm,
            )
            # resall[:, m] = lsum + dterm = lnsum - (B-i)*S_ii
            nc.vector.tensor_add(resall[:, m:m + 1], lsum, dterm)

        nc.sync.dma_start(out=out, in_=resall)

    nc.compile()
    return nc


def _get_nc():
    if "nc" not in _CACHE:
        _CACHE["nc"] = _build()
    return _CACHE["nc"]


def kernel(x: np.ndarray, y: np.ndarray) -> np.ndarray:
    global LAST_RESULTS
    from concourse import bass_utils

    nc = _get_nc()

    x = np.asarray(x, dtype=np.float32)
    y = np.asarray(y, dtype=np.float32)

    yT_full = np.ascontiguousarray(y.T.astype(ml_dtypes.bfloat16))  # [D, B]
    nhits = (B - np.arange(B, dtype=np.float64)) / TEMP             # (B-i)/T
    in_maps = []
    for c in range(NCORES):
        sl = slice(c * ROWS, (c + 1) * ROWS)
        xs = x[sl]                                                   # [ROWS, D]
        in_maps.append(
            {
                "xT": np.ascontiguousarray(xs.T.astype(ml_dtypes.bfloat16)),
                "yT": yT_full,
                "xpre": np.ascontiguousarray(
                    (-nhits[sl, None] * xs.astype(np.float64)).astype(np.float32)
                ),
                "ysh": np.ascontiguousarray(y[sl]),
            }
        )

    res = bass_utils.run_bass_kernel_spmd(
        nc, in_maps, core_ids=list(range(NCORES))
    )
    LAST_RESULTS = res

    total = 0.0
    for c in range(NCORES):
        total += res.results[c]["partial"].astype(np.float64).sum()
    return np.asarray(total, dtype=np.float32)
